# revision 57
# baseline (speedup 1.0000x reference)
"""BitNet Llama MLP on 8 trn2 NeuronCores (Bass/Tile).

y = bitlinear(silu(bitlinear(x, w_gate)) * bitlinear(x, w_up), w_down)

Strategy:
  * All fake-quantized values are small integers -> exact in bf16. The three
    matmuls run as bf16 integer matmuls accumulating exactly in fp32 PSUM;
    dequant scales are applied afterwards.
  * I (11008) padded to 11264 = 8*1408; gate/up are tensor-parallel over I.
    The down-proj is token-parallel (1024 tokens/core): the quantized
    intermediate h is resharded with bf16 AllToAlls instead of all-reducing a
    134MB fp32 output.
  * Quantized x is int8 (exact) -> transposed on-chip, ONE 4.2MB/rank int8
    AllGather (halved bytes, one collective slot); converted back to bf16
    per token-block in phase B on the Activation+Vector engines.
  * Weight scales (mean|w|): per-shard |w| row-sums on the Activation engine
    (abs + accum_out) while the Vector engine quantizes x; one 32B
    AllReduce. The last xqT stripe store is data-anchored to the broadcast
    scale (no-op min vs 127) so the scale AllReduce always reaches the
    collective core before the xq AllGather, unblocking weight quantization
    at the earliest possible point.
  * wgT/wuT stored in [i-tile][p][a][128] layout: contiguous 8KB/partition
    runs on both the phase-A store and the per-(tb,i) phase-B reload.
  * w_down^T AllGathered in 4 bf16 chunks. Chunk c's readiness is anchored
    to token-block c's last habs reduce (deferred stripe store + no-op min),
    and its trigger is emitted two blocks later, so on the collective queue
    each chunk lands right AFTER that block's habs AllReduce: the habs ARs
    (which gate the h-quant pipeline) are never stuck behind a
    multi-hundred-us AG, and the AGs fill the gaps between them.
  * h absmax over the sharded I dim: elementwise-max accumulation, PE
    transpose partition-reduction, then a per-token-block AllReduce(max);
    ReduceScatter(max) of the same buffer gives phase C its dequant scales.
    The h-quant pipeline runs two token-blocks behind gate/up compute.
  * The AllToAll is split per 512-local-token half. The LAST token block is
    processed sub-half-major with per-half habs AllReduces, so its first
    half's h-quant and the first AllToAll both run UNDER the second half's
    gate/up matmuls; the down matmul on half 0 then starts right after the
    last block ends, and half 1's AllToAll hides under half 0's matmuls.
  * The last h-quant half runs out of a small persistent pool so the B-scope
    pool close (which hands SBUF to phase C) never waits on it: phase C's
    hq/wd loads start the moment the last gate/up matmul retires.

TimelineSim: 4.609 ms vs 7.179 ms for the previous version (PE busy 3.74 ms
= 81%; rel_l2 vs reference 9.07e-4, identical to the previous version).
"""

import sys

sys.path.insert(0, "/opt/trn_rl_repo")

import numpy as np

import concourse.bass as bass
import concourse.bacc as bacc
import concourse.mybir as mybir
import concourse.tile as tile
from concourse.bass_utils import run_bass_kernel_spmd
from concourse.masks import make_identity

F32 = mybir.dt.float32
BF16 = mybir.dt.bfloat16
I8 = mybir.dt.int8
MAGIC = 12582912.0  # 1.5*2^23: fp32 add/sub rounds to nearest int (ties even)
EPS = 1e-5
N_CORES = 8

FULL_CFG = dict(H=4096, Tc=1024, Ish=1408, count=11008 * 4096)


def build_program(H, Tc, Ish, count):
    T = Tc * N_CORES
    n_ht = H // 128
    n_it = Ish // 128
    n_itot = N_CORES * n_it
    n_xt = Tc // 128
    SUBS = Tc // 512
    N_WC = 4                  # w_down AG chunks
    WCW = H // N_WC           # 1024 cols per chunk
    rg = [list(range(N_CORES))]
    AX = mybir.AxisListType.X
    OP = mybir.AluOpType
    ACT = mybir.ActivationFunctionType

    nc = bacc.Bacc("TRN2", target_bir_lowering=False, debug=False,
                   num_devices=N_CORES)

    x_s = nc.dram_tensor("x_s", [Tc, H], F32, kind="ExternalInput")
    wg_s = nc.dram_tensor("wg_s", [Ish, H], F32, kind="ExternalInput")
    wu_s = nc.dram_tensor("wu_s", [Ish, H], F32, kind="ExternalInput")
    wd_s = nc.dram_tensor("wd_s", [H, Ish], F32, kind="ExternalInput")
    y = nc.dram_tensor("y", [Tc, H], F32, kind="ExternalOutput")

    with tile.TileContext(nc) as tc:
        with (
            tc.tile_pool(name="const", bufs=1) as cpool,
            tc.tile_pool(name="pql", bufs=1) as pql,
            tc.tile_pool(name="dram", bufs=1, space="DRAM") as dram,
        ):
            # ---------------- DRAM intermediates ----------------
            xqT_s = dram.tile([H, Tc], I8, tag="xqT_s")
            xqT_q = dram.tile([N_CORES, H, Tc], I8, tag="xqT_q",
                              addr_space="Shared")
            deq_s = dram.tile([1, Tc], F32, tag="deq_s")
            deq_all = dram.tile([N_CORES, Tc], F32, tag="deq_all",
                                addr_space="Shared")
            wgT_q = dram.tile([n_it, 128, n_ht, 128], BF16, tag="wgT_q")
            wuT_q = dram.tile([n_it, 128, n_ht, 128], BF16, tag="wuT_q")
            wdT_q = dram.tile([N_WC, Ish, WCW], BF16, tag="wdT_q")
            wdT_all = [dram.tile([N_CORES, Ish, WCW], BF16, tag=f"wdT_all{c}",
                                 name=f"wdT_all{c}", addr_space="Shared")
                       for c in range(N_WC)]
            ar_in = dram.tile([1, 8], F32, tag="ar_in")
            ar_out = dram.tile([1, 8], F32, tag="ar_out", addr_space="Shared")
            h_send = dram.tile([N_CORES, Ish, Tc], F32, tag="h_send")
            a2a_send = dram.tile([SUBS, N_CORES, Ish, 512], BF16,
                                 tag="a2a_send")
            a2a_recv = dram.tile([SUBS, N_CORES, Ish, 512], BF16,
                                 tag="a2a_recv")
            habs_part = dram.tile([1, T], F32, tag="habs_part")
            habs_all = [dram.tile([1, Tc], F32, tag=f"habs_all{b}",
                                  name=f"habs_all{b}", addr_space="Shared")
                        for b in range(N_CORES)]
            # tb=7 uses per-half AllReduces (Shared tiles allow one writer)
            habs_half = [dram.tile([1, 512], F32, tag=f"habs_h{s}",
                                   name=f"habs_h{s}", addr_space="Shared")
                         for s in range(2)]
            habs_rs = dram.tile([1, Tc], F32, tag="habs_rs")

            idb = cpool.tile([128, 128], BF16, tag="idb")
            make_identity(nc, idb[:])
            idf = cpool.tile([128, 128], F32, tag="idf")
            make_identity(nc, idf[:])
            scB = cpool.tile([128, 8], F32, tag="scB")   # sw_g, sw_u, sw_d
            rcB = cpool.tile([128, 8], F32, tag="rcB")   # 1/sw
            magicB = cpool.tile([128, 1], F32, tag="magicB")
            nc.vector.memset(magicB[:], MAGIC)
            zeroB = cpool.tile([128, 1], F32, tag="zeroB")
            nc.vector.memset(zeroB[:], 0.0)
            # per-chunk stash for one deferred wdT stripe; see emit_b
            wd_stash = [cpool.tile([128, Ish // 128, 128], BF16,
                                   tag=f"wdst{c}", name=f"wdst{c}")
                        for c in range(N_WC)]

            # ================= PHASE A =================
            with (
                tc.tile_pool(name="pa", bufs=2) as pa,
                tc.tile_pool(name="pa1", bufs=1) as pa1,
                tc.tile_pool(name="paps", bufs=7, space="PSUM") as paps,
                tc.tile_pool(name="paps1", bufs=1, space="PSUM") as paps1,
            ):
                # ---- A1: weight |w| partial sums + scale AllReduce ----
                acc = pa1.tile([128, 8], F32, tag="acc")
                nc.vector.memset(acc[:], 0.0)
                accP = pa1.tile([128, 8], F32, tag="accP")
                nc.gpsimd.memset(accP[:], 0.0)
                srcs = [(0, wg_s[it * 128:(it + 1) * 128, :], f"g{it}")
                        for it in range(n_it)]
                srcs += [(1, wu_s[it * 128:(it + 1) * 128, :], f"u{it}")
                         for it in range(n_it)]
                srcs += [(2, wd_s[ht * 128:(ht + 1) * 128, :], f"d{ht}")
                         for ht in range(n_ht)]
                def emit_x_tile(tt):
                    xt = pa.tile([128, H], F32, tag="af32a", name=f"xt{tt}")
                    nc.sync.dma_start(xt[:], x_s[tt * 128:(tt + 1) * 128, :])
                    amax = pa.tile([128, 1], F32, tag="rsm", name=f"am{tt}")
                    nc.vector.tensor_reduce(amax[:], xt[:], axis=AX, op=OP.max,
                                            apply_absolute_value=True)
                    amc = pa.tile([128, 1], F32, tag="amc", name=f"amc{tt}")
                    nc.vector.tensor_scalar_max(amc[:], amax[:], EPS)
                    deq = pa.tile([128, 1], F32, tag="deq", name=f"dq{tt}")
                    nc.vector.tensor_scalar_mul(deq[:], amc[:], 1.0 / 127.0)
                    nc.sync.dma_start(deq_s[0, tt * 128:(tt + 1) * 128], deq[:])
                    rec = pa.tile([128, 1], F32, tag="rec", name=f"rc{tt}")
                    nc.vector.reciprocal(rec[:], amc[:])
                    qs = pa.tile([128, 1], F32, tag="qs", name=f"qsc{tt}")
                    nc.vector.tensor_scalar_mul(qs[:], rec[:], 127.0)
                    t0 = pa.tile([128, H], F32, tag="af32b", name=f"t0_{tt}")
                    nc.vector.tensor_scalar(t0[:], xt[:], qs[:], MAGIC,
                                            op0=OP.mult, op1=OP.add)
                    t1 = pa.tile([128, H], F32, tag="af32c", name=f"t1_{tt}")
                    nc.vector.tensor_scalar(t1[:], t0[:], MAGIC, 127.0,
                                            op0=OP.subtract, op1=OP.min)
                    qb = pa.tile([128, H], BF16, tag="abf", name=f"qb{tt}")
                    nc.vector.tensor_scalar_max(qb[:], t1[:], -128.0)
                    last = (tt == n_xt - 1)
                    xqs = pa.tile([128, n_ht, 128], I8,
                                  tag="asm8L" if last else "asm8",
                                  name=f"xqs{tt}")
                    for ht in range(n_ht):
                        tp = paps.tile([128, 128], BF16, tag="tpb",
                                       name=f"xtp{tt}_{ht}")
                        nc.tensor.transpose(tp[:], qb[:, ht * 128:(ht + 1) * 128],
                                            idb[:])
                        if ht % 2 == 0:
                            nc.vector.tensor_copy(xqs[:, ht, :], tp[:])
                        else:
                            nc.scalar.copy(xqs[:, ht, :], tp[:])
                    if last:
                        xqs_hold.append(xqs)
                    else:
                        nc.sync.dma_start(
                            xqT_s.rearrange("(a p) t -> p a t", p=128)[
                                :, :, tt * 128:(tt + 1) * 128],
                            xqs[:])

                xqs_hold = []
                # interleave one x tile after every 7 weight-abs tiles
                next_x = 0
                for si, (j, src, nm) in enumerate(srcs):
                    if si % 7 == 0 and next_x < n_xt:
                        emit_x_tile(next_x)
                        next_x += 1
                    wt = pa.tile([128, H], F32, tag="aw32", name=f"wab{nm}")
                    nc.sync.dma_start(wt[:, :src.shape[1]], src)
                    r = pa.tile([128, 1], F32, tag="rsw", name=f"rab{nm}")
                    nc.scalar.activation(wt[:, :src.shape[1]],
                                         wt[:, :src.shape[1]],
                                         ACT.Abs, accum_out=r[:])
                    nc.gpsimd.tensor_tensor(accP[:, j:j + 1],
                                            accP[:, j:j + 1],
                                            r[:], op=OP.add)
                while next_x < n_xt:
                    emit_x_tile(next_x)
                    next_x += 1
                nc.vector.tensor_tensor(acc[:], acc[:], accP[:], op=OP.add)
                tacc = paps1.tile([8, 128], F32, tag="tacc")
                nc.tensor.transpose(tacc[:], acc[:], idf[:])
                asum = pa1.tile([8, 1], F32, tag="asum")
                nc.vector.tensor_reduce(asum[:], tacc[:], axis=AX, op=OP.add)
                nc.sync.dma_start(ar_in[0, 0:8], asum[:])
                nc.gpsimd.collective_compute(
                    "AllReduce", OP.add, replica_groups=rg,
                    ins=[ar_in[:]], outs=[ar_out[:]])
                ars = pa1.tile([1, 8], F32, tag="ars")
                nc.sync.dma_start(ars[:], ar_out[:])
                sc1 = pa1.tile([1, 8], F32, tag="sc1")
                nc.vector.tensor_scalar(sc1[:], ars[:], 1.0 / count, EPS,
                                        op0=OP.mult, op1=OP.max)
                rc1 = pa1.tile([1, 8], F32, tag="rc1")
                nc.vector.reciprocal(rc1[:], sc1[:])
                nc.gpsimd.partition_broadcast(scB[:], sc1[:])
                nc.gpsimd.partition_broadcast(rcB[:], rc1[:])
                # anchor the last xqs store to the scale broadcast (no-op
                # min vs 127) so the xq AllGather can only become ready
                # after the scale AllReduce has run: AR always wins the
                # collective core, unblocking weight quantization early
                xqs7 = xqs_hold[0]
                d127 = pa1.tile([128, 1], I8, tag="d127")
                nc.vector.tensor_scalar(d127[:], scB[:, 0:1], 0.0, 127.0,
                                        op0=OP.mult, op1=OP.add)
                nc.vector.tensor_tensor(xqs7[:, 0, 0:1], xqs7[:, 0, 0:1],
                                        d127[:], op=OP.min)
                nc.sync.dma_start(
                    xqT_s.rearrange("(a p) t -> p a t", p=128)[
                        :, :, (n_xt - 1) * 128:n_xt * 128],
                    xqs7[:])

                # ---- A2 tail: xq + deq AllGathers (x tiles emitted above) ---
                nc.gpsimd.collective_compute(
                    "AllGather", OP.bypass, replica_groups=rg,
                    ins=[xqT_s[:]], outs=[xqT_q[:]])
                nc.gpsimd.collective_compute(
                    "AllGather", OP.bypass, replica_groups=rg,
                    ins=[deq_s[:]], outs=[deq_all[:]])

                # ---- A3: weight quant + transpose (wg/wu first, wd after) ---
                def quant_weight_tile(src_ap, j, nm, width):
                    wt = pa.tile([128, width], F32, tag="aw32", name=f"wqi{nm}")
                    nc.sync.dma_start(wt[:], src_ap)
                    q0 = pa.tile([128, width], F32, tag="af32b", name=f"wq0{nm}")
                    nc.vector.tensor_scalar(q0[:], wt[:], rcB[:, j:j + 1], MAGIC,
                                            op0=OP.mult, op1=OP.add)
                    q1 = pa.tile([128, width], F32, tag="af32c", name=f"wq1{nm}")
                    nc.vector.tensor_scalar(q1[:], q0[:], MAGIC, 1.0,
                                            op0=OP.subtract, op1=OP.min)
                    qq = pa.tile([128, width], BF16, tag="abf", name=f"wqq{nm}")
                    nc.vector.tensor_scalar_max(qq[:], q1[:], -1.0)
                    return qq

                for j, (w, wT) in enumerate(((wg_s, wgT_q), (wu_s, wuT_q))):
                    for it in range(n_it):
                        qq = quant_weight_tile(w[it * 128:(it + 1) * 128, :], j,
                                               f"{j}_{it}", H)
                        ws = pa.tile([128, n_ht, 128], BF16, tag="asm",
                                     name=f"wqs{j}_{it}")
                        for ht in range(n_ht):
                            tp = paps.tile([128, 128], BF16, tag="tpb",
                                           name=f"wtp{j}_{it}_{ht}")
                            nc.tensor.transpose(tp[:], qq[:, ht * 128:(ht + 1) * 128], idb[:])
                            if ht % 2 == 0:
                                nc.vector.tensor_copy(ws[:, ht, :], tp[:])
                            else:
                                nc.scalar.copy(ws[:, ht, :], tp[:])
                        nc.sync.dma_start(wT[it], ws[:])

                n_wht = WCW // 128   # 8 h-tiles per wd chunk
                for ht in range(n_ht):
                    qq = quant_weight_tile(wd_s[ht * 128:(ht + 1) * 128, :], 2,
                                           f"d_{ht}", Ish)
                    deferred = (ht % n_wht == 0)
                    if deferred:
                        ws = wd_stash[ht // n_wht]
                    else:
                        ws = pa.tile([128, n_it, 128], BF16, tag="asm",
                                     name=f"wdqs{ht}")
                    for it in range(n_it):
                        tp = paps.tile([128, 128], BF16, tag="tpb",
                                       name=f"dtp{ht}_{it}")
                        nc.tensor.transpose(tp[:], qq[:, it * 128:(it + 1) * 128], idb[:])
                        if it % 2 == 0:
                            nc.vector.tensor_copy(ws[:, it, :], tp[:])
                        else:
                            nc.scalar.copy(ws[:, it, :], tp[:])
                    if not deferred:
                        nc.sync.dma_start(
                            wdT_q.rearrange("c (a p) h -> c p a h", p=128)[
                                ht // n_wht, :, :,
                                (ht % n_wht) * 128:(ht % n_wht + 1) * 128],
                            ws[:])
            # ================= PHASE B: gate/up + pipelined h quant =========
            with (
                tc.tile_pool(name="pbx8", bufs=1) as pbx8,
                tc.tile_pool(name="pbx", bufs=1) as pbx,
                tc.tile_pool(name="pbw", bufs=2) as pbw,
                tc.tile_pool(name="pbe", bufs=2) as pbe,
                tc.tile_pool(name="pbm", bufs=1) as pbm,
                tc.tile_pool(name="pq", bufs=3) as pq,
                tc.tile_pool(name="pq1", bufs=2) as pq1,
                tc.tile_pool(name="pbps", bufs=3, space="PSUM") as pbps,
                tc.tile_pool(name="pbpt", bufs=2, space="PSUM") as pbpt,
            ):
                xqT_v = xqT_q.rearrange("b (a p) t -> b p a t", p=128)

                def emit_mm_block(tb, i, sub, wg_t, wu_t, xq_sub, dg_bt, du_bt,
                                  maxacc):
                    sl = slice(sub * 512, (sub + 1) * 512)
                    ps_g = pbps.tile([128, 512], F32, tag="ps_g",
                                     name=f"psg{tb}_{i}_{sub}")
                    for k in range(n_ht):
                        nc.tensor.matmul(ps_g[:], wg_t[:, k, :],
                                         xq_sub[sub][:, k, :],
                                         start=(k == 0), stop=(k == n_ht - 1))
                    ps_u = pbps.tile([128, 512], F32, tag="ps_u",
                                     name=f"psu{tb}_{i}_{sub}")
                    for k in range(n_ht):
                        nc.tensor.matmul(ps_u[:], wu_t[:, k, :],
                                         xq_sub[sub][:, k, :],
                                         start=(k == 0), stop=(k == n_ht - 1))
                    g = pbe.tile([128, 512], F32, tag="g", name=f"g{tb}_{i}_{sub}")
                    nc.vector.tensor_tensor(g[:], ps_g[:], dg_bt[:, sl], op=OP.mult)
                    sg = pbe.tile([128, 512], F32, tag="sg", name=f"sg{tb}_{i}_{sub}")
                    nc.scalar.activation(sg[:], g[:], ACT.Silu)
                    u = pbe.tile([128, 512], F32, tag="u", name=f"u{tb}_{i}_{sub}")
                    nc.vector.tensor_tensor(u[:], ps_u[:], du_bt[:, sl], op=OP.mult)
                    h = pbe.tile([128, 512], F32, tag="h", name=f"h{tb}_{i}_{sub}")
                    nc.vector.tensor_tensor(h[:], sg[:], u[:], op=OP.mult)
                    nc.sync.dma_start(
                        h_send[tb, i * 128:(i + 1) * 128, sl], h[:])
                    if i == 0:
                        nc.vector.scalar_tensor_tensor(
                            maxacc[:, sl], h[:], -1.0, h[:],
                            op0=OP.mult, op1=OP.max)
                    else:
                        ha = pbe.tile([128, 512], F32, tag="ha",
                                      name=f"ha{tb}_{i}_{sub}")
                        nc.vector.scalar_tensor_tensor(
                            ha[:], h[:], -1.0, h[:], op0=OP.mult, op1=OP.max)
                        nc.vector.tensor_tensor(maxacc[:, sl], maxacc[:, sl],
                                                ha[:], op=OP.max)

                def emit_b(tb, split=False):
                    # chunk tb-2's AG: its readiness was anchored to B(tb-2)'s
                    # last habs reduce (just after ARh(tb-2)'s input), so the
                    # collective core runs ARh(tb-2) first, then this AG;
                    # triggering it two blocks later keeps the data-wait
                    # already satisfied so Pool.SEQ never blocks on it
                    if 2 <= tb <= N_WC + 1:
                        nc.gpsimd.collective_compute(
                            "AllGather", OP.bypass, replica_groups=rg,
                            ins=[wdT_q[tb - 2]], outs=[wdT_all[tb - 2][:]])
                    dq_row = pbm.tile([1, Tc], F32, tag="dqrow", name=f"dqr{tb}")
                    nc.sync.dma_start(dq_row[:], deq_all[tb:tb + 1, :])
                    dq_bt = pbm.tile([128, Tc], F32, tag="dqbt", name=f"dqb{tb}")
                    nc.gpsimd.partition_broadcast(dq_bt[:], dq_row[:])
                    dg_bt = pbm.tile([128, Tc], F32, tag="dgbt", name=f"dgb{tb}")
                    nc.vector.tensor_scalar_mul(dg_bt[:], dq_bt[:], scB[:, 0:1])
                    du_bt = dq_bt
                    nc.vector.tensor_scalar_mul(du_bt[:], du_bt[:], scB[:, 1:2])
                    maxacc = pbm.tile([128, Tc], F32, tag="maxacc", name=f"mx{tb}")
                    xq_sub = []
                    for sub in range(SUBS):
                        xi8 = pbx8.tile([128, n_ht, 512], I8, tag="xi8",
                                        name=f"xi8{tb}_{sub}")
                        nc.sync.dma_start(
                            xi8[:], xqT_v[tb][:, :, sub * 512:(sub + 1) * 512])
                        xq = pbx.tile([128, n_ht, 512], BF16, tag=f"xq{sub}",
                                      name=f"xq{tb}_{sub}")
                        half = n_ht // 2
                        nc.scalar.copy(xq[:, :half, :], xi8[:, :half, :])
                        nc.vector.tensor_copy(xq[:, half:, :], xi8[:, half:, :])
                        xq_sub.append(xq)
                    def habs_quarters(qlo, qhi):
                        rl = None
                        for q in range(qlo, qhi):
                            tp = pbpt.tile([128, 128], F32, tag="tpf",
                                           name=f"mtp{tb}_{q}")
                            nc.tensor.transpose(
                                tp[:], maxacc[:, q * 128:(q + 1) * 128], idf[:])
                            red = pbe.tile([128, 1], F32, tag="red",
                                           name=f"red{tb}_{q}")
                            nc.vector.tensor_reduce(red[:], tp[:], axis=AX,
                                                    op=OP.max)
                            rl = red
                            nc.sync.dma_start(
                                habs_part[0, tb * Tc + q * 128:
                                          tb * Tc + (q + 1) * 128],
                                red[:])
                        return rl

                    if split:
                        # sub-major order: sub-0's habs AR, h-quant, and the
                        # first AllToAll all fire while sub-1 still computes
                        for sub in range(SUBS):
                            for i in range(n_it):
                                wg_t = pbw.tile([128, n_ht, 128], BF16,
                                                tag="wg_t",
                                                name=f"wg{tb}_{sub}_{i}")
                                nc.sync.dma_start(wg_t[:], wgT_q[i])
                                wu_t = pbw.tile([128, n_ht, 128], BF16,
                                                tag="wu_t",
                                                name=f"wu{tb}_{sub}_{i}")
                                nc.sync.dma_start(wu_t[:], wuT_q[i])
                                emit_mm_block(tb, i, sub, wg_t, wu_t, xq_sub,
                                              dg_bt, du_bt, maxacc)
                            habs_quarters(sub * 4, sub * 4 + 4)
                            ssl = slice(tb * Tc + sub * 512,
                                        tb * Tc + (sub + 1) * 512)
                            nc.gpsimd.collective_compute(
                                "AllReduce", OP.max, replica_groups=rg,
                                ins=[habs_part[0:1, ssl]],
                                outs=[habs_half[sub][:]])
                            emit_qh_half(tb, sub, last=(sub == SUBS - 1))
                            if sub == 0:
                                nc.gpsimd.collective_compute(
                                    "AllToAll", OP.bypass, replica_groups=rg,
                                    ins=[a2a_send[0]], outs=[a2a_recv[0]])
                        return
                    for i in range(n_it):
                        wg_t = pbw.tile([128, n_ht, 128], BF16, tag="wg_t",
                                        name=f"wg{tb}_{i}")
                        nc.sync.dma_start(wg_t[:], wgT_q[i])
                        wu_t = pbw.tile([128, n_ht, 128], BF16, tag="wu_t",
                                        name=f"wu{tb}_{i}")
                        nc.sync.dma_start(wu_t[:], wuT_q[i])
                        for sub in range(SUBS):
                            emit_mm_block(tb, i, sub, wg_t, wu_t, xq_sub,
                                          dg_bt, du_bt, maxacc)
                    red_last = habs_quarters(0, Tc // 128)
                    nc.gpsimd.collective_compute(
                        "AllReduce", OP.max, replica_groups=rg,
                        ins=[habs_part[0:1, tb * Tc:(tb + 1) * Tc]],
                        outs=[habs_all[tb][:]])
                    if tb < N_WC:
                        # anchor chunk-tb readiness just after ARh(tb)'s
                        # inputs: a no-op min (ternary values vs >= 2)
                        # data-dependent on the last habs reduce, then the
                        # deferred stripe store completes the chunk
                        dep = pbe.tile([128, 1], F32, tag="dep",
                                       name=f"dep{tb}")
                        nc.vector.tensor_scalar(dep[:], red_last[:], 0.0, 2.0,
                                                op0=OP.mult, op1=OP.add)
                        nc.vector.tensor_tensor(
                            wd_stash[tb][:, 0, 0:1], wd_stash[tb][:, 0, 0:1],
                            dep[:], op=OP.min)
                        nc.sync.dma_start(
                            wdT_q.rearrange("c (a p) h -> c p a h", p=128)[
                                tb, :, :, 0:128],
                            wd_stash[tb][:])

                def emit_qh_half(tb, sub, last=False):
                    sl = slice(sub * 512, (sub + 1) * 512)
                    if last:
                        def htile(shape, dt, tag, name):
                            return pql.tile(shape, dt, tag=tag, name=name)
                    else:
                        def htile(shape, dt, tag, name):
                            pool = pq1 if tag in ("hrow", "qsbt") else pq
                            return pool.tile(shape, dt, tag=tag, name=name)
                    hrow = htile([1, 512], F32, "hrow", f"hrH{tb}_{sub}")
                    nc.sync.dma_start(hrow[:], habs_half[sub][:])
                    qs_bt = htile([128, 512], F32, "qsbt", f"qsH{tb}_{sub}")
                    nc.gpsimd.partition_broadcast(qs_bt[:], hrow[:])
                    nc.vector.tensor_scalar_max(qs_bt[:], qs_bt[:], EPS)
                    nc.vector.reciprocal(qs_bt[:], qs_bt[:])
                    nc.vector.tensor_scalar_mul(qs_bt[:], qs_bt[:], 127.0)
                    for i in range(n_it):
                        hl = htile([128, 512], F32,
                                   f"hl{i % 2}" if last else "hl",
                                   f"hlH{tb}_{sub}_{i}")
                        nc.sync.dma_start(
                            hl[:], h_send[tb, i * 128:(i + 1) * 128, sl])
                        nc.vector.tensor_tensor(hl[:], hl[:], qs_bt[:],
                                                op=OP.mult)
                        nc.vector.tensor_scalar(hl[:], hl[:], MAGIC, MAGIC,
                                                op0=OP.add, op1=OP.subtract)
                        qb = htile([128, 512], BF16,
                                   f"qb{i % 2}" if last else "qb2",
                                   f"hqH{tb}_{sub}_{i}")
                        nc.vector.tensor_scalar(qb[:], hl[:], 127.0, -128.0,
                                                op0=OP.min, op1=OP.max)
                        nc.sync.dma_start(
                            a2a_send[sub, tb, i * 128:(i + 1) * 128, :],
                            qb[:])

                def emit_qh(tb):
                    hrow = pq1.tile([1, Tc], F32, tag="hrow", name=f"hr{tb}")
                    nc.sync.dma_start(hrow[:], habs_all[tb][:])
                    qs_bt = pq1.tile([128, Tc], F32, tag="qsbt", name=f"qsb{tb}")
                    nc.gpsimd.partition_broadcast(qs_bt[:], hrow[:])
                    nc.vector.tensor_scalar_max(qs_bt[:], qs_bt[:], EPS)
                    nc.vector.reciprocal(qs_bt[:], qs_bt[:])
                    nc.vector.tensor_scalar_mul(qs_bt[:], qs_bt[:], 127.0)
                    for i in range(n_it):
                        hl = pq.tile([128, Tc], F32, tag="hl", name=f"hl{tb}_{i}")
                        nc.sync.dma_start(hl[:], h_send[tb, i * 128:(i + 1) * 128, :])
                        nc.vector.tensor_tensor(hl[:], hl[:], qs_bt[:], op=OP.mult)
                        nc.vector.tensor_scalar(hl[:], hl[:], MAGIC, MAGIC,
                                                op0=OP.add, op1=OP.subtract)
                        qb = pq.tile([128, Tc], BF16, tag="qb2", name=f"hqb{tb}_{i}")
                        nc.vector.tensor_scalar(qb[:], hl[:], 127.0, -128.0,
                                                op0=OP.min, op1=OP.max)
                        for s in range(SUBS):
                            nc.sync.dma_start(
                                a2a_send[s, tb, i * 128:(i + 1) * 128, :],
                                qb[:, s * 512:(s + 1) * 512])

                for tb in range(N_CORES - 1):
                    emit_b(tb)
                    if tb >= 2:
                        emit_qh(tb - 2)
                emit_qh(N_CORES - 3)
                emit_qh(N_CORES - 2)
                emit_b(N_CORES - 1, split=True)

            nc.gpsimd.collective_compute(
                "ReduceScatter", OP.max, replica_groups=rg,
                ins=[habs_part[:]], outs=[habs_rs[:]])
            nc.gpsimd.collective_compute(
                "AllToAll", OP.bypass, replica_groups=rg,
                ins=[a2a_send[1]], outs=[a2a_recv[1]])

            # ================= PHASE C: down =================
            hq_v = a2a_recv.rearrange("s b (a p) t -> s p b a t", p=128)
            wd_vs = [w.rearrange("b (a p) h -> p b a h", p=128)
                     for w in wdT_all]
            n_hcw = WCW // 512   # 512-col hc blocks per wd chunk
            with (
                tc.tile_pool(name="pch", bufs=1) as pch,
                tc.tile_pool(name="pcw", bufs=2) as pcw,
                tc.tile_pool(name="pcd", bufs=2) as pcd,
                tc.tile_pool(name="pcps", bufs=2, space="PSUM") as pcps,
            ):
                for half in range(Tc // 512):
                    hq_t = pch.tile([128, n_itot, 512], BF16, tag="hq_t",
                                    name=f"hq{half}")
                    nc.sync.dma_start(hq_t[:], hq_v[half])
                    dv_all = pcd.tile([128, 4], F32, tag="dv", name=f"dv{half}")
                    for tt in range(4):
                        dvl = pcd.tile([128, 1], F32, tag="dvl", name=f"dvl{half}_{tt}")
                        nc.sync.dma_start(
                            dvl[:],
                            habs_rs[0, half * 512 + tt * 128: half * 512 + (tt + 1) * 128])
                        dvc = pcd.tile([128, 1], F32, tag="dvc", name=f"dvc{half}_{tt}")
                        nc.vector.tensor_scalar_max(dvc[:], dvl[:], EPS)
                        nc.vector.tensor_scalar(dv_all[:, tt:tt + 1], dvc[:],
                                                scB[:, 2:3], 1.0 / 127.0,
                                                op0=OP.mult, op1=OP.mult)
                    for hc in range(H // 512):
                        wv = wd_vs[hc // n_hcw]
                        off = (hc % n_hcw) * 512
                        pss = []
                        for cc in range(2):
                            wd_t = pcw.tile([128, n_itot, 256], BF16, tag="wd_t",
                                            name=f"wd{half}_{hc}_{cc}")
                            nc.sync.dma_start(
                                wd_t[:],
                                wv[:, :, :, off + cc * 256:off + (cc + 1) * 256])
                            for tt in range(4):
                                if cc == 0:
                                    ps = pcps.tile([128, 512], F32, tag=f"psy{tt}",
                                                   name=f"psy{half}_{hc}_{tt}")
                                    pss.append(ps)
                                ps = pss[tt]
                                for ii in range(n_itot):
                                    nc.tensor.matmul(
                                        ps[:, cc * 256:(cc + 1) * 256],
                                        hq_t[:, ii, tt * 128:(tt + 1) * 128],
                                        wd_t[:, ii, :],
                                        start=(ii == 0), stop=(ii == n_itot - 1))
                        for tt in range(4):
                            yv = pcd.tile([128, 512], F32, tag="yv",
                                          name=f"yv{half}_{hc}_{tt}")
                            nc.vector.tensor_scalar_mul(yv[:], pss[tt][:],
                                                        dv_all[:, tt:tt + 1])
                            nc.sync.dma_start(
                                y[half * 512 + tt * 128: half * 512 + (tt + 1) * 128,
                                  hc * 512:(hc + 1) * 512],
                                yv[:])

    nc.compile()
    return nc


_CACHE = {}


def _get_program():
    if "full" not in _CACHE:
        _CACHE["full"] = build_program(**FULL_CFG)
    return _CACHE["full"]


def kernel(x, w_gate, w_up, w_down):
    B, S, H = x.shape
    I = w_gate.shape[0]
    T = B * S
    Tc = T // N_CORES
    Ish = FULL_CFG["Ish"]
    Ipad = Ish * N_CORES

    xf = np.ascontiguousarray(np.asarray(x, np.float32).reshape(T, H))
    wg_pad = np.zeros((Ipad, H), np.float32)
    wg_pad[:I] = np.asarray(w_gate, np.float32)
    wu_pad = np.zeros((Ipad, H), np.float32)
    wu_pad[:I] = np.asarray(w_up, np.float32)
    wd_pad = np.zeros((H, Ipad), np.float32)
    wd_pad[:, :I] = np.asarray(w_down, np.float32)

    in_maps = []
    for c in range(N_CORES):
        in_maps.append({
            "x_s": np.ascontiguousarray(xf[c * Tc:(c + 1) * Tc]),
            "wg_s": np.ascontiguousarray(wg_pad[c * Ish:(c + 1) * Ish]),
            "wu_s": np.ascontiguousarray(wu_pad[c * Ish:(c + 1) * Ish]),
            "wd_s": np.ascontiguousarray(wd_pad[:, c * Ish:(c + 1) * Ish]),
        })

    nc = _get_program()
    res = run_bass_kernel_spmd(nc, in_maps, core_ids=list(range(N_CORES)))
    out = np.concatenate([res.results[c]["y"] for c in range(N_CORES)], axis=0)
    return out.reshape(B, S, H).astype(np.float32)
